# revision 23
# baseline (speedup 1.0000x reference)
"""CosineAttention on 8 TRN2 NeuronCores — v3 (batch-sequential).

Sharding (head-parallel): core c owns head h=c for both batches.

Per-core pipeline (all-bf16 datapath, fp32 PSUM accum):
  stage 0: load full xT [512, 4096] bf16 into SBUF, DMAs spread over
    two queues so the first chunk lands fast.
  stage 1 (4 chunks of 1024 tokens):
    - qkT = wqk^T-stationary matmuls; sq = qk^2 on ACT (Square, PSUM src)
    - st = per-token sum of squares via ones-matmul
    - r' = rsqrt(st*scl) via ACT Abs_reciprocal_sqrt; scl folds the /64
      mean and the 1/8 attention scale (into the q row only)
    - r' broadcast across partitions via an e2 outer-product matmul,
      ACT-copied to SBUF; DVE muls read qk straight from PSUM
    - qn lands on partitions 0-63; kn is DMA-shifted from partitions
      64-127 down to 0-63 (DVE lanes cannot cross partitions)
    - v: wv-stationary vT matmuls + XBAR DMA transpose into [j, d]
  stage 2, batches SEQUENTIAL (av needs only 2 PSUM banks at a time,
  which frees 6 banks = 3 rotating sim tiles so the PE can run ahead
  and ACT exp — the true bottleneck — stays saturated):
    for b: for ic (i-chunks of 1024): for jt (16 j-tiles):
      simT -> ACT Exp -> expT bf16; AV matmuls pipelined one jt behind
    finalize per (b, ic): 1/se via DVE reciprocal_approx_fast (se copied
    to partition 0 first), GPSIMD partition broadcast, DVE mul -> oc
    AllGather per (b, ic) [64, 1024] bf16 so batch 0's collectives and
    out-projection overlap batch 1's attention; only the last AG tails.
  out-proj per (b, ic): w2^T-stationary matmuls on gathered [512, 1024].
"""

import os
os.environ.setdefault("NEURON_RT_DBG_RDH_CC", "0")

import numpy as np
import ml_dtypes

import concourse.bass as bass
import concourse.tile as tile
from concourse import bacc
import concourse.mybir as mybir
from concourse import bass_utils

f32 = mybir.dt.float32
f32r = mybir.dt.float32r
bf16 = mybir.dt.bfloat16
AF = mybir.ActivationFunctionType

N_CORES = 8
HEADS = 8
D = 64            # head dim
B = 2             # batch
SEQ = 2048        # tokens per batch
DIM = 512         # model dim
NTOK = B * SEQ    # 4096
SCALE = D ** -0.5

FT = DIM // 128   # 4 feature tiles
CH = 1024         # stage-1 token chunk
NCH = NTOK // CH  # 4
IC = 1024         # stage-2 i-chunk
NIC = SEQ // IC   # 2
JT = SEQ // 128   # 16 j-tiles per batch

DEBUG_DUMP = False

_BUILD_CACHE = {}


def _emit_outproj(nc, sb, ps, cc_out, w2_sb, outT, b, ic):
    ag_sb = sb.tile([128, FT, IC], bf16, tag="ag", name=f"ag{b}_{ic}")
    for t in range(FT):
        nc.sync.dma_start(ag_sb[:, t, :], cc_out[t * 128:(t + 1) * 128, :])
    fp_ps = ps.tile([D, IC], f32, tag="sim", bufs=3, name=f"fp{b}_{ic}")
    for h in range(IC // 512):
        hc = slice(h * 512, (h + 1) * 512)
        for t in range(FT):
            nc.tensor.matmul(fp_ps[:, hc], w2_sb[:, t, :], ag_sb[:, t, hc],
                             start=(t == 0), stop=(t == FT - 1))
    fo_sb = sb.tile([D, IC], f32, tag="fo", name=f"fo{b}_{ic}")
    nc.vector.tensor_copy(fo_sb[:], fp_ps[:])
    nc.sync.dma_start(
        outT[:, b * SEQ + ic * IC:b * SEQ + (ic + 1) * IC], fo_sb[:])


def build(num_devices=N_CORES):
    key = (num_devices,)
    if key in _BUILD_CACHE:
        return _BUILD_CACHE[key]
    nc = bacc.Bacc("TRN2", target_bir_lowering=False, debug=False,
                   num_devices=num_devices)
    xT = nc.dram_tensor("xT", [DIM, NTOK], bf16, kind="ExternalInput").ap()
    wqk = nc.dram_tensor("wqk", [DIM, 128], bf16, kind="ExternalInput").ap()
    wv = nc.dram_tensor("wv", [DIM, D], bf16, kind="ExternalInput").ap()
    w2 = nc.dram_tensor("w2", [DIM, D], bf16, kind="ExternalInput").ap()
    o2 = nc.dram_tensor("o2", [128, 2], bf16, kind="ExternalInput").ap()
    scl = nc.dram_tensor("scl", [2, 1], f32, kind="ExternalInput").ap()
    e2 = nc.dram_tensor("e2", [2, 128], f32, kind="ExternalInput").ap()
    o1 = nc.dram_tensor("o1", [1, D], mybir.dt.float16,
                        kind="ExternalInput").ap()
    outT = nc.dram_tensor("outT", [D, NTOK], f32, kind="ExternalOutput").ap()
    if DEBUG_DUMP:
        dbg_qn = nc.dram_tensor("dbg_qn", [D, NTOK], f32,
                                kind="ExternalOutput").ap()
        dbg_kn = nc.dram_tensor("dbg_kn", [D, NTOK], f32,
                                kind="ExternalOutput").ap()
        dbg_vo = nc.dram_tensor("dbg_vo", [128, 2 * JT * (D + 1)], f32,
                                kind="ExternalOutput").ap()
        dbg_ex = nc.dram_tensor("dbg_ex", [128, IC], f32,
                                kind="ExternalOutput").ap()
        dbg_av = nc.dram_tensor("dbg_av", [D + 1, IC], f32,
                                kind="ExternalOutput").ap()

    with tile.TileContext(nc) as tc:
        with (
            tc.tile_pool(name="persist", bufs=1) as pp,
            tc.tile_pool(name="sb", bufs=2) as sb,
            tc.tile_pool(name="ps", bufs=1, space="PSUM") as ps,
            tc.tile_pool(name="dram", bufs=1, space="DRAM") as dram,
            nc.allow_low_precision(reason="bf16 datapath; approx reciprocal"),
        ):
            # ---- persistent weights / constants ----
            wqk_sb = pp.tile([128, FT, 128], bf16)
            wv_sb = pp.tile([128, FT, D], bf16)
            w2_sb = pp.tile([128, FT, D], bf16)
            for t in range(FT):
                rows = slice(t * 128, (t + 1) * 128)
                nc.scalar.dma_start(wqk_sb[:, t, :], wqk[rows, :])
                nc.scalar.dma_start(wv_sb[:, t, :], wv[rows, :])
                nc.scalar.dma_start(w2_sb[:, t, :], w2[rows, :])
            o2_sb = pp.tile([128, 2], bf16)
            nc.scalar.dma_start(o2_sb[:], o2[:])
            scl_sb = pp.tile([2, 1], f32)
            nc.scalar.dma_start(scl_sb[:], scl[:])
            e2_sb = pp.tile([2, 128], f32r)
            nc.scalar.dma_start(e2_sb[:], e2[:].bitcast(f32r))
            o1_sb = pp.tile([1, D], mybir.dt.float16)
            nc.scalar.dma_start(o1_sb[:], o1[:])

            # ---- full xT resident in SBUF; two DMA queues, chunk-major ----
            xt_sb = pp.tile([128, FT, NTOK], bf16)
            engs = [nc.sync, nc.gpsimd, nc.scalar]
            for c in range(NCH):
                cols = slice(c * CH, (c + 1) * CH)
                for t in range(FT):
                    rows = slice(t * 128, (t + 1) * 128)
                    eng = engs[(c * FT + t) % 3]
                    eng.dma_start(xt_sb[:, t, cols], xT[rows, cols])

            qn_sb = pp.tile([D, NTOK], bf16)
            kn_sb = pp.tile([D, NTOK], bf16)
            vo_sb = pp.tile([128, 2 * JT, D + 1], bf16)  # v | ones
            nc.gpsimd.memset(vo_sb[:, :, D:D + 1], 1.0)

            # ---- stage 1: projections + cosine normalization ----
            # all qk chains first so the ACT rsqrt work (and its table set)
            # finishes before the first exp; v matmuls follow on the PE
            for c in range(NCH):
                qk_ps = ps.tile([128, CH], f32, tag="sim", bufs=3)
                for h in range(CH // 512):
                    hc = slice(h * 512, (h + 1) * 512)
                    xcol = slice(c * CH + h * 512, c * CH + (h + 1) * 512)
                    for t in range(FT):
                        nc.tensor.matmul(qk_ps[:, hc], wqk_sb[:, t, :],
                                         xt_sb[:, t, xcol],
                                         start=(t == 0), stop=(t == FT - 1))
                sq_sb = sb.tile([128, CH], bf16, tag="sq")
                nc.scalar.activation(sq_sb[:], qk_ps[:], AF.Square)
                st_ps = ps.tile([2, CH], f32, tag="sim", bufs=3)
                for h in range(CH // 512):
                    hc = slice(h * 512, (h + 1) * 512)
                    nc.tensor.matmul(st_ps[:, hc], o2_sb[:], sq_sb[:, hc],
                                     start=True, stop=True)
                # r'[0] = rsqrt(st_q) = rsqrt(ms_q)/8, r'[1] = rsqrt(ms_k)
                rt_sb = sb.tile([2, CH], f32r, tag="rt")
                nc.scalar.activation(rt_sb[:], st_ps[:],
                                     AF.Abs_reciprocal_sqrt, scale=scl_sb[:])
                r_ps = ps.tile([128, CH], f32, tag="sim", bufs=3)
                for h in range(CH // 512):
                    hc = slice(h * 512, (h + 1) * 512)
                    nc.tensor.matmul(r_ps[:, hc], e2_sb[:], rt_sb[:, hc],
                                     start=True, stop=True)
                rb_sb = sb.tile([128, CH], f32, tag="rb")
                nc.scalar.activation(rb_sb[:], r_ps[:], AF.Copy)
                dcol = slice(c * CH, (c + 1) * CH)
                nc.vector.tensor_mul(qn_sb[:, dcol], qk_ps[0:64, :],
                                     rb_sb[0:64, :])
                kh_sb = sb.tile([128, CH], bf16, tag="kh")
                nc.vector.tensor_mul(kh_sb[64:128, :], qk_ps[64:128, :],
                                     rb_sb[64:128, :])
                nc.gpsimd.dma_start(kn_sb[:, dcol], kh_sb[64:128, :])

            for c in range(NCH):
                # v: wv-stationary vT then XBAR DMA transpose into [j, d]
                vt_ps = ps.tile([D, CH], f32, tag="sim", bufs=3)
                for h in range(CH // 512):
                    hc = slice(h * 512, (h + 1) * 512)
                    xcol = slice(c * CH + h * 512, c * CH + (h + 1) * 512)
                    for t in range(FT):
                        nc.tensor.matmul(vt_ps[:, hc], wv_sb[:, t, :],
                                         xt_sb[:, t, xcol],
                                         start=(t == 0), stop=(t == FT - 1))
                vt_sb = sb.tile([D, CH], bf16, tag="vt")
                nc.vector.tensor_copy(vt_sb[:], vt_ps[:])
                vj_sb = sb.tile([128, CH // 128, D], bf16, tag="vj")
                nc.sync.dma_start_transpose(vj_sb[:], vt_sb[:])
                nc.vector.tensor_copy(vo_sb[:, c * 8:(c + 1) * 8, 0:D],
                                      vj_sb[:])

            if DEBUG_DUMP:
                for nm, t_sb, dst in (("qn", qn_sb, dbg_qn),
                                      ("kn", kn_sb, dbg_kn)):
                    d_f = sb.tile([D, NTOK], f32, tag=f"dbg{nm}", bufs=1)
                    nc.vector.tensor_copy(d_f[:], t_sb[:])
                    nc.sync.dma_start(dst[:], d_f[:])
                dv = sb.tile([128, 2 * JT, D + 1], f32, tag="dbgv", bufs=1)
                nc.vector.tensor_copy(dv[:], vo_sb[:])
                nc.sync.dma_start(dbg_vo[:], dv[:])

            # ---- stage 2: attention, batches sequential ----
            b0_cc = None
            for b in range(B):
                cc_pairs = []
                for ic in range(NIC):
                    cc_in = dram.tile([D, IC], bf16, name=f"cc_in{b}_{ic}")
                    cc_out = dram.tile([DIM, IC], bf16, addr_space="Shared",
                                       name=f"cc_out{b}_{ic}")
                    cc_pairs.append((cc_in, cc_out))
                    av = ps.tile([D + 1, IC], f32, tag="av", bufs=1,
                                 name=f"av{b}_{ic}")
                    ex_prev = None
                    for jt in range(JT + 1):
                        ex_cur = None
                        if jt < JT:
                            sim_ps = ps.tile([128, IC], f32, tag="sim",
                                             bufs=3, name="sim")
                            for h in range(IC // 512):
                                hc = slice(h * 512, (h + 1) * 512)
                                nc.tensor.matmul(
                                    sim_ps[:, hc],
                                    kn_sb[:, b * SEQ + jt * 128:
                                          b * SEQ + (jt + 1) * 128],
                                    qn_sb[:, b * SEQ + ic * IC + h * 512:
                                          b * SEQ + ic * IC + (h + 1) * 512],
                                    start=True, stop=True)
                            ex_cur = sb.tile([128, IC], bf16, tag="ex",
                                             bufs=6, name="ex")
                            nc.scalar.activation(ex_cur[:], sim_ps[:], AF.Exp)
                            if DEBUG_DUMP and b == 0 and ic == 0 and jt == 0:
                                de = sb.tile([128, IC], f32, tag="dbge",
                                             bufs=1)
                                nc.vector.tensor_copy(de[:], ex_cur[:])
                                nc.sync.dma_start(dbg_ex[:], de[:])
                        if jt > 0:
                            pjt = jt - 1
                            for h in range(IC // 512):
                                hc = slice(h * 512, (h + 1) * 512)
                                nc.tensor.matmul(av[:, hc],
                                                 vo_sb[:, b * JT + pjt, :],
                                                 ex_prev[:, hc],
                                                 start=(pjt == 0),
                                                 stop=(pjt == JT - 1))
                        ex_prev = ex_cur
                        # overlap batch-0 out-projection with batch-1
                        # attention once its AllGathers have landed
                        if b == 1 and ic == 1 and jt == 6:
                            _emit_outproj(nc, sb, ps, b0_cc[0][1], w2_sb,
                                          outT, 0, 0)
                        if b == 1 and ic == 1 and jt == 10:
                            _emit_outproj(nc, sb, ps, b0_cc[1][1], w2_sb,
                                          outT, 0, 1)

                    if DEBUG_DUMP and b == 0 and ic == 0:
                        da = sb.tile([D + 1, IC], f32, tag="dbga", bufs=1)
                        nc.vector.tensor_copy(da[:], av[:])
                        nc.sync.dma_start(dbg_av[:], da[:])
                    # finalize: oc = av * (1/se). No gpsimd here — it may
                    # be blocked waiting on an in-flight AllGather; the
                    # broadcast is a PE outer product instead.
                    se0_sb = sb.tile([1, IC], f32, tag="se0")
                    nc.vector.tensor_copy(se0_sb[:], av[D:D + 1, :])
                    rse_sb = sb.tile([1, IC], f32, tag="rse")
                    nc.vector.reciprocal_approx_fast(rse_sb[:], se0_sb[:])
                    rse16_sb = sb.tile([1, IC], mybir.dt.float16, tag="rse16")
                    nc.vector.tensor_copy(rse16_sb[:], rse_sb[:])
                    av_sb = sb.tile([D, IC], bf16, tag="avs")
                    nc.vector.tensor_copy(av_sb[:], av[0:D, :])
                    rb2_ps = ps.tile([D, IC], f32, tag="sim", bufs=3,
                                     name="rb2")
                    for h in range(IC // 512):
                        hc = slice(h * 512, (h + 1) * 512)
                        nc.tensor.matmul(rb2_ps[:, hc], o1_sb[:],
                                         rse16_sb[:, hc],
                                         start=True, stop=True)
                    oc_sb = sb.tile([D, IC], bf16, tag="oc")
                    nc.vector.tensor_mul(oc_sb[:], av_sb[:], rb2_ps[:])
                    nc.sync.dma_start(cc_in[:], oc_sb[:])
                    nc.gpsimd.collective_compute(
                        "AllGather", mybir.AluOpType.bypass,
                        replica_groups=[list(range(num_devices))],
                        ins=[cc_in[:]], outs=[cc_out[:]])
                if b == 0:
                    b0_cc = cc_pairs

            # ---- out-projection for batch 1 (batch 0 was interleaved) ----
            for ic in range(NIC):
                _emit_outproj(nc, sb, ps, cc_pairs[ic][1], w2_sb, outT, 1, ic)
    nc.compile()
    _BUILD_CACHE[key] = nc
    return nc


def make_in_maps(x, Wq, Wkv, Wout):
    to_bf = lambda a: np.ascontiguousarray(a).astype(ml_dtypes.bfloat16)
    xT = to_bf(x.reshape(NTOK, DIM).T)
    o2 = np.zeros((128, 2), np.float32)
    o2[0:D, 0] = 1.0
    o2[D:128, 1] = 1.0
    o2 = o2.astype(ml_dtypes.bfloat16)
    scl = np.array([[1.0], [1.0 / D]], np.float32)
    e2 = np.zeros((2, 128), np.float32)
    e2[0, 0:D] = 1.0
    e2[1, D:128] = 1.0
    o1 = np.ones((1, D), np.float16)
    in_maps = []
    for c in range(N_CORES):
        rows = slice(c * D, (c + 1) * D)
        wqk = to_bf(np.concatenate([Wq[rows, :].T, Wkv[rows, :].T], axis=1))
        wv = to_bf(Wkv[DIM + c * D:DIM + (c + 1) * D, :].T)
        w2 = to_bf(Wout[rows, :].T)
        in_maps.append({
            "xT": xT, "wqk": wqk, "wv": wv, "w2": w2, "o2": o2, "scl": scl,
            "e2": e2, "o1": o1,
        })
    return in_maps


def kernel(x, Wq, Wkv, Wout, _trace=False):
    nc = build()
    in_maps = make_in_maps(np.asarray(x), np.asarray(Wq), np.asarray(Wkv),
                           np.asarray(Wout))
    res = bass_utils.run_bass_kernel_spmd(
        nc, in_maps, core_ids=list(range(N_CORES)), trace=_trace)
    out = np.empty((NTOK, DIM), np.float32)
    for c in range(N_CORES):
        out[:, c * D:(c + 1) * D] = res.results[c]["outT"].T
    full = out.reshape(B, SEQ, DIM)
    if _trace:
        return full, res
    return full


# revision 27
# speedup vs baseline: 1.0257x; 1.0257x over previous
"""CosineAttention on 8 TRN2 NeuronCores — v3 (batch-sequential).

Sharding (head-parallel): core c owns head h=c for both batches.

Per-core pipeline (all-bf16 datapath, fp32 PSUM accum):
  stage 0: load full xT [512, 4096] bf16 into SBUF, DMAs spread over
    two queues so the first chunk lands fast.
  stage 1 (4 chunks of 1024 tokens):
    - qkT = wqk^T-stationary matmuls; sq = qk^2 on ACT (Square, PSUM src)
    - st = per-token sum of squares via ones-matmul
    - r' = rsqrt(st*scl) via ACT Abs_reciprocal_sqrt; scl folds the /64
      mean and the 1/8 attention scale (into the q row only)
    - r' broadcast across partitions via an e2 outer-product matmul,
      ACT-copied to SBUF; DVE muls read qk straight from PSUM
    - qn lands on partitions 0-63; kn is DMA-shifted from partitions
      64-127 down to 0-63 (DVE lanes cannot cross partitions)
    - v: wv-stationary vT matmuls + XBAR DMA transpose into [j, d]
  stage 2, batches SEQUENTIAL (av needs only 2 PSUM banks at a time,
  which frees 6 banks = 3 rotating sim tiles so the PE can run ahead
  and ACT exp — the true bottleneck — stays saturated):
    for b: for ic (i-chunks of 1024): for jt (16 j-tiles):
      simT -> ACT Exp -> expT bf16; AV matmuls pipelined one jt behind
    finalize per (b, ic): 1/se via DVE reciprocal_approx_fast (se copied
    to partition 0 first), GPSIMD partition broadcast, DVE mul -> oc
    AllGather per (b, ic) [64, 1024] bf16 so batch 0's collectives and
    out-projection overlap batch 1's attention; only the last AG tails.
  out-proj per (b, ic): w2^T-stationary matmuls on gathered [512, 1024].
"""

import numpy as np
import ml_dtypes

import concourse.bass as bass
import concourse.tile as tile
from concourse import bacc
import concourse.mybir as mybir
from concourse import bass_utils

f32 = mybir.dt.float32
f32r = mybir.dt.float32r
bf16 = mybir.dt.bfloat16
AF = mybir.ActivationFunctionType

N_CORES = 8
HEADS = 8
D = 64            # head dim
B = 2             # batch
SEQ = 2048        # tokens per batch
DIM = 512         # model dim
NTOK = B * SEQ    # 4096
SCALE = D ** -0.5

FT = DIM // 128   # 4 feature tiles
CH = 1024         # stage-1 token chunk
NCH = NTOK // CH  # 4
IC = 1024         # stage-2 i-chunk
NIC = SEQ // IC   # 2
JT = SEQ // 128   # 16 j-tiles per batch

DEBUG_DUMP = False

_BUILD_CACHE = {}


def _emit_outproj(nc, sb, ps, cc_out, w2_sb, outT, b, ic):
    ag_sb = sb.tile([128, FT, IC], bf16, tag="ag", name=f"ag{b}_{ic}")
    for t in range(FT):
        for hf in range(2):
            nc.sync.dma_start(ag_sb[:, t, hf * 512:(hf + 1) * 512],
                              cc_out[hf][t * 128:(t + 1) * 128, :])
    fp_ps = ps.tile([D, IC], f32, tag="sim", bufs=3, name=f"fp{b}_{ic}")
    for h in range(IC // 512):
        hc = slice(h * 512, (h + 1) * 512)
        for t in range(FT):
            nc.tensor.matmul(fp_ps[:, hc], w2_sb[:, t, :], ag_sb[:, t, hc],
                             start=(t == 0), stop=(t == FT - 1))
    fo_sb = sb.tile([D, IC], f32, tag="fo", name=f"fo{b}_{ic}")
    nc.vector.tensor_copy(fo_sb[:], fp_ps[:])
    nc.sync.dma_start(
        outT[:, b * SEQ + ic * IC:b * SEQ + (ic + 1) * IC], fo_sb[:])


def build(num_devices=N_CORES):
    key = (num_devices,)
    if key in _BUILD_CACHE:
        return _BUILD_CACHE[key]
    nc = bacc.Bacc("TRN2", target_bir_lowering=False, debug=False,
                   num_devices=num_devices)
    xT = nc.dram_tensor("xT", [DIM, NTOK], bf16, kind="ExternalInput").ap()
    wqk = nc.dram_tensor("wqk", [DIM, 128], bf16, kind="ExternalInput").ap()
    wv = nc.dram_tensor("wv", [DIM, D], bf16, kind="ExternalInput").ap()
    w2 = nc.dram_tensor("w2", [DIM, D], bf16, kind="ExternalInput").ap()
    o2 = nc.dram_tensor("o2", [128, 2], bf16, kind="ExternalInput").ap()
    scl = nc.dram_tensor("scl", [2, 1], f32, kind="ExternalInput").ap()
    e2 = nc.dram_tensor("e2", [2, 128], f32, kind="ExternalInput").ap()
    o1 = nc.dram_tensor("o1", [1, D], mybir.dt.float16,
                        kind="ExternalInput").ap()
    outT = nc.dram_tensor("outT", [D, NTOK], f32, kind="ExternalOutput").ap()
    if DEBUG_DUMP:
        dbg_qn = nc.dram_tensor("dbg_qn", [D, NTOK], f32,
                                kind="ExternalOutput").ap()
        dbg_kn = nc.dram_tensor("dbg_kn", [D, NTOK], f32,
                                kind="ExternalOutput").ap()
        dbg_vo = nc.dram_tensor("dbg_vo", [128, 2 * JT * (D + 1)], f32,
                                kind="ExternalOutput").ap()
        dbg_ex = nc.dram_tensor("dbg_ex", [128, IC], f32,
                                kind="ExternalOutput").ap()
        dbg_av = nc.dram_tensor("dbg_av", [D + 1, IC], f32,
                                kind="ExternalOutput").ap()

    with tile.TileContext(nc) as tc:
        with (
            tc.tile_pool(name="persist", bufs=1) as pp,
            tc.tile_pool(name="sb", bufs=2) as sb,
            tc.tile_pool(name="ps", bufs=1, space="PSUM") as ps,
            tc.tile_pool(name="dram", bufs=1, space="DRAM") as dram,
            nc.allow_low_precision(reason="bf16 datapath; approx reciprocal"),
        ):
            # ---- persistent weights / constants ----
            wqk_sb = pp.tile([128, FT, 128], bf16)
            wv_sb = pp.tile([128, FT, D], bf16)
            w2_sb = pp.tile([128, FT, D], bf16)
            for t in range(FT):
                rows = slice(t * 128, (t + 1) * 128)
                nc.scalar.dma_start(wqk_sb[:, t, :], wqk[rows, :])
                nc.scalar.dma_start(wv_sb[:, t, :], wv[rows, :])
                nc.scalar.dma_start(w2_sb[:, t, :], w2[rows, :])
            o2_sb = pp.tile([128, 2], bf16)
            nc.scalar.dma_start(o2_sb[:], o2[:])
            scl_sb = pp.tile([2, 1], f32)
            nc.scalar.dma_start(scl_sb[:], scl[:])
            e2_sb = pp.tile([2, 128], f32r)
            nc.scalar.dma_start(e2_sb[:], e2[:].bitcast(f32r))
            o1_sb = pp.tile([1, D], mybir.dt.float16)
            nc.scalar.dma_start(o1_sb[:], o1[:])

            # ---- full xT resident in SBUF; two DMA queues, chunk-major ----
            xt_sb = pp.tile([128, FT, NTOK], bf16)
            engs = [nc.sync, nc.gpsimd, nc.scalar]
            for c in range(NCH):
                cols = slice(c * CH, (c + 1) * CH)
                for t in range(FT):
                    rows = slice(t * 128, (t + 1) * 128)
                    eng = engs[(c * FT + t) % 3]
                    eng.dma_start(xt_sb[:, t, cols], xT[rows, cols])

            qn_sb = pp.tile([D, NTOK], bf16)
            kn_sb = pp.tile([D, NTOK], bf16)
            vo_sb = pp.tile([128, 2 * JT, D + 1], bf16)  # v | ones
            nc.gpsimd.memset(vo_sb[:, :, D:D + 1], 1.0)

            # ---- stage 1: projections + cosine normalization ----
            # all qk chains first so the ACT rsqrt work (and its table set)
            # finishes before the first exp; v matmuls follow on the PE
            for c in range(NCH):
                qk_ps = ps.tile([128, CH], f32, tag="sim", bufs=3)
                for h in range(CH // 512):
                    hc = slice(h * 512, (h + 1) * 512)
                    xcol = slice(c * CH + h * 512, c * CH + (h + 1) * 512)
                    for t in range(FT):
                        nc.tensor.matmul(qk_ps[:, hc], wqk_sb[:, t, :],
                                         xt_sb[:, t, xcol],
                                         start=(t == 0), stop=(t == FT - 1))
                sq_sb = sb.tile([128, CH], bf16, tag="sq")
                nc.scalar.activation(sq_sb[:], qk_ps[:], AF.Square)
                st_ps = ps.tile([2, CH], f32, tag="sim", bufs=3)
                for h in range(CH // 512):
                    hc = slice(h * 512, (h + 1) * 512)
                    nc.tensor.matmul(st_ps[:, hc], o2_sb[:], sq_sb[:, hc],
                                     start=True, stop=True)
                # r'[0] = rsqrt(st_q) = rsqrt(ms_q)/8, r'[1] = rsqrt(ms_k)
                rt_sb = sb.tile([2, CH], f32r, tag="rt")
                nc.scalar.activation(rt_sb[:], st_ps[:],
                                     AF.Abs_reciprocal_sqrt, scale=scl_sb[:])
                r_ps = ps.tile([128, CH], f32, tag="sim", bufs=3)
                for h in range(CH // 512):
                    hc = slice(h * 512, (h + 1) * 512)
                    nc.tensor.matmul(r_ps[:, hc], e2_sb[:], rt_sb[:, hc],
                                     start=True, stop=True)
                rb_sb = sb.tile([128, CH], f32, tag="rb")
                nc.scalar.activation(rb_sb[:], r_ps[:], AF.Copy)
                dcol = slice(c * CH, (c + 1) * CH)
                nc.vector.tensor_mul(qn_sb[:, dcol], qk_ps[0:64, :],
                                     rb_sb[0:64, :])
                kh_sb = sb.tile([128, CH], bf16, tag="kh")
                nc.vector.tensor_mul(kh_sb[64:128, :], qk_ps[64:128, :],
                                     rb_sb[64:128, :])
                nc.gpsimd.dma_start(kn_sb[:, dcol], kh_sb[64:128, :])

            for c in range(NCH):
                # v directly in [j, d] layout: xT-slice stationary (bf16 FWL
                # keeps the per-tile LDWEIGHTS cheap)
                for g in range(2):
                    v_ps = ps.tile([128, 4, D], f32, tag="sim", bufs=3,
                                   name="vps")
                    for jj in range(4):
                        jw = slice(c * CH + (4 * g + jj) * 128,
                                   c * CH + (4 * g + jj) * 128 + 128)
                        for t in range(FT):
                            nc.tensor.matmul(
                                v_ps[:, jj, :], xt_sb[:, t, jw],
                                wv_sb[:, t, :],
                                start=(t == 0), stop=(t == FT - 1))
                    jt0 = c * 8 + g * 4
                    nc.vector.tensor_copy(vo_sb[:, jt0:jt0 + 4, 0:D],
                                          v_ps[:])

            if DEBUG_DUMP:
                for nm, t_sb, dst in (("qn", qn_sb, dbg_qn),
                                      ("kn", kn_sb, dbg_kn)):
                    d_f = sb.tile([D, NTOK], f32, tag=f"dbg{nm}", bufs=1)
                    nc.vector.tensor_copy(d_f[:], t_sb[:])
                    nc.sync.dma_start(dst[:], d_f[:])
                dv = sb.tile([128, 2 * JT, D + 1], f32, tag="dbgv", bufs=1)
                nc.vector.tensor_copy(dv[:], vo_sb[:])
                nc.sync.dma_start(dbg_vo[:], dv[:])

            # ---- stage 2: attention, batches sequential ----
            b0_cc = None
            for b in range(B):
                cc_pairs = []
                for ic in range(NIC):
                    cc_in = [dram.tile([D, IC // 2], bf16,
                                       name=f"cc_in{b}_{ic}_{hf}")
                             for hf in range(2)]
                    cc_out = [dram.tile([DIM, IC // 2], bf16,
                                        addr_space="Shared",
                                        name=f"cc_out{b}_{ic}_{hf}")
                              for hf in range(2)]
                    cc_pairs.append((cc_in, cc_out))
                    av = ps.tile([D + 1, IC], f32, tag="av", bufs=1,
                                 name=f"av{b}_{ic}")
                    ex_prev = None
                    for jt in range(JT + 1):
                        ex_cur = None
                        if jt < JT:
                            sim_ps = ps.tile([128, IC], f32, tag="sim",
                                             bufs=3, name="sim")
                            for h in range(IC // 512):
                                hc = slice(h * 512, (h + 1) * 512)
                                nc.tensor.matmul(
                                    sim_ps[:, hc],
                                    kn_sb[:, b * SEQ + jt * 128:
                                          b * SEQ + (jt + 1) * 128],
                                    qn_sb[:, b * SEQ + ic * IC + h * 512:
                                          b * SEQ + ic * IC + (h + 1) * 512],
                                    start=True, stop=True)
                            ex_cur = sb.tile([128, IC], bf16, tag="ex",
                                             bufs=6, name="ex")
                            nc.scalar.activation(ex_cur[:], sim_ps[:], AF.Exp)
                            if DEBUG_DUMP and b == 0 and ic == 0 and jt == 0:
                                de = sb.tile([128, IC], f32, tag="dbge",
                                             bufs=1)
                                nc.vector.tensor_copy(de[:], ex_cur[:])
                                nc.sync.dma_start(dbg_ex[:], de[:])
                        if jt > 0:
                            pjt = jt - 1
                            for h in range(IC // 512):
                                hc = slice(h * 512, (h + 1) * 512)
                                nc.tensor.matmul(av[:, hc],
                                                 vo_sb[:, b * JT + pjt, :],
                                                 ex_prev[:, hc],
                                                 start=(pjt == 0),
                                                 stop=(pjt == JT - 1))
                        ex_prev = ex_cur
                        # overlap batch-0 out-projection with batch-1
                        # attention once its AllGathers have landed
                        if b == 1 and ic == 1 and jt == 6:
                            _emit_outproj(nc, sb, ps, b0_cc[0][1], w2_sb,
                                          outT, 0, 0)
                        if b == 1 and ic == 1 and jt == 10:
                            _emit_outproj(nc, sb, ps, b0_cc[1][1], w2_sb,
                                          outT, 0, 1)

                    if DEBUG_DUMP and b == 0 and ic == 0:
                        da = sb.tile([D + 1, IC], f32, tag="dbga", bufs=1)
                        nc.vector.tensor_copy(da[:], av[:])
                        nc.sync.dma_start(dbg_av[:], da[:])
                    # finalize: oc = av * (1/se). No gpsimd here — it may
                    # be blocked waiting on an in-flight AllGather; the
                    # broadcast is a PE outer product instead.
                    se0_sb = sb.tile([1, IC], f32, tag="se0")
                    nc.vector.tensor_copy(se0_sb[:], av[D:D + 1, :])
                    rse_sb = sb.tile([1, IC], f32, tag="rse")
                    nc.vector.reciprocal_approx_fast(rse_sb[:], se0_sb[:])
                    rse16_sb = sb.tile([1, IC], mybir.dt.float16, tag="rse16")
                    nc.vector.tensor_copy(rse16_sb[:], rse_sb[:])
                    av_sb = sb.tile([D, IC], bf16, tag="avs")
                    nc.vector.tensor_copy(av_sb[:], av[0:D, :])
                    rb2_ps = ps.tile([D, IC], f32, tag="sim", bufs=3,
                                     name="rb2")
                    for h in range(IC // 512):
                        hc = slice(h * 512, (h + 1) * 512)
                        nc.tensor.matmul(rb2_ps[:, hc], o1_sb[:],
                                         rse16_sb[:, hc],
                                         start=True, stop=True)
                    oc_sb = sb.tile([D, IC], bf16, tag="oc")
                    nc.vector.tensor_mul(oc_sb[:], av_sb[:], rb2_ps[:])
                    for hf in range(2):
                        nc.sync.dma_start(cc_in[hf][:],
                                          oc_sb[:, hf * 512:(hf + 1) * 512])
                    # two half-chunk AllGathers (64 KB in, 512 KB out) stay
                    # under the mesh-algorithm threshold; the final one
                    # triggers from scalar (exp work is over) so it does not
                    # chain behind gpsimd's previous AG completion wait
                    for hf in range(2):
                        nc.gpsimd.collective_compute(
                            "AllGather", mybir.AluOpType.bypass,
                            replica_groups=[list(range(num_devices))],
                            ins=[cc_in[hf][:]], outs=[cc_out[hf][:]])
                if b == 0:
                    b0_cc = cc_pairs

            # ---- out-projection for batch 1 (batch 0 was interleaved) ----
            for ic in range(NIC):
                _emit_outproj(nc, sb, ps, cc_pairs[ic][1], w2_sb, outT, 1, ic)
    nc.compile()
    _BUILD_CACHE[key] = nc
    return nc


def make_in_maps(x, Wq, Wkv, Wout):
    to_bf = lambda a: np.ascontiguousarray(a).astype(ml_dtypes.bfloat16)
    xT = to_bf(x.reshape(NTOK, DIM).T)
    o2 = np.zeros((128, 2), np.float32)
    o2[0:D, 0] = 1.0
    o2[D:128, 1] = 1.0
    o2 = o2.astype(ml_dtypes.bfloat16)
    scl = np.array([[1.0], [1.0 / D]], np.float32)
    e2 = np.zeros((2, 128), np.float32)
    e2[0, 0:D] = 1.0
    e2[1, D:128] = 1.0
    o1 = np.ones((1, D), np.float16)
    in_maps = []
    for c in range(N_CORES):
        rows = slice(c * D, (c + 1) * D)
        wqk = to_bf(np.concatenate([Wq[rows, :].T, Wkv[rows, :].T], axis=1))
        wv = to_bf(Wkv[DIM + c * D:DIM + (c + 1) * D, :].T)
        w2 = to_bf(Wout[rows, :].T)
        in_maps.append({
            "xT": xT, "wqk": wqk, "wv": wv, "w2": w2, "o2": o2, "scl": scl,
            "e2": e2, "o1": o1,
        })
    return in_maps


def kernel(x, Wq, Wkv, Wout, _trace=False):
    nc = build()
    in_maps = make_in_maps(np.asarray(x), np.asarray(Wq), np.asarray(Wkv),
                           np.asarray(Wout))
    res = bass_utils.run_bass_kernel_spmd(
        nc, in_maps, core_ids=list(range(N_CORES)), trace=_trace)
    out = np.empty((NTOK, DIM), np.float32)
    for c in range(N_CORES):
        out[:, c * D:(c + 1) * D] = res.results[c]["outT"].T
    full = out.reshape(B, SEQ, DIM)
    if _trace:
        return full, res
    return full
